# revision 1
# baseline (speedup 1.0000x reference)
# Trainium2 Bass kernel for nn_DTIHarmonicIS (DTI_PDBbind-style GAT + pairwise
# harmonic interaction energies). Data-parallel over batch B=8 across 8 cores.
#
# Self-contained: hardcodes all shapes/sharding. kernel(**inputs) takes FULL
# inputs (as produced by setup_inputs) and returns the FULL [B, 7] output.

import numpy as np

import concourse.bass as bass
import concourse.bacc as bacc
import concourse.tile as tile
import concourse.mybir as mybir
from concourse.alu_op_type import AluOpType
from concourse.bass_utils import run_bass_kernel_spmd

B, N1, N2, D, L, H, NT = 8, 64, 512, 128, 3, 128, 7
F_IN = 56
DM_MIN = 0.5
BIG = 1000.0  # softmax mask offset; masked entries underflow to exact 0 in exp
B_CONSTRAINT = np.array([1.159, 0.448, 0.927, 0.902, 0.349, 0.789, 0.198],
                        np.float32)
BC_INV = (1.0 / (3.0 * B_CONSTRAINT ** 2)).astype(np.float32)

f32 = mybir.dt.float32
AF = mybir.ActivationFunctionType
AX = mybir.AxisListType

# Fraction of pairwise relu units routed to the ACT engine (rest go to DVE).
import os as _os
ACT_RELU_FRAC = float(_os.environ.get('KFRAC', '0.22'))

import os
STAGE = int(os.environ.get('KSTAGE', '3'))  # 1=loads+dm+final, 2=+GAT, 3=full
LOOP_N = int(os.environ.get('KLOOP', '1'))  # >1: repeat body in-NEFF (timing)
PAIR_BF16 = os.environ.get('KBF16', '0') == '1'  # bf16 relu/matvec datapath
TRACE = False           # unused here (no NTFF hook in this environment)
TIMING_REPS = 0         # set >0 (e.g. from test.py) to wall-clock repeat runs
LAST_RESULT = {}        # timing info stashed here after each run

_cache = {}


def _build():
    nc = bacc.Bacc("TRN2", target_bir_lowering=False)

    def inp(name, shape):
        return nc.dram_tensor(name, shape, f32, kind="ExternalInput")

    # per-core (batch-sliced) data
    t_h1T = inp("h1T", [F_IN, N1])
    t_h2T = inp("h2T", [F_IN, N2])
    t_adj1T = inp("adj1T", [N1, N1])
    t_adj2T = inp("adj2T", [N2, N2])
    t_Aint = inp("A_intT", [NT, N2, N1])
    t_dmv = inp("dmvT", [N2, N1 * 3])
    t_valid = inp("valid", [N1, 1])
    t_sum4 = inp("sum4", [4 * NT, NT])
    # weights (replicated across cores)
    t_Wemb = inp("W_embed", [F_IN, D])
    t_gW = inp("gW", [L, D, D])
    t_gA = inp("gA", [L, D, D])
    t_gWb = inp("gWbT", [D, L])
    t_gGW = inp("gGateW_s", [D, L, 2])
    t_gGb = inp("gGateb_r", [1, L])
    t_WA1 = inp("WA1_s", [NT, 2, D, H])
    t_WB1 = inp("WB1_s", [NT, 2, D, H])
    t_bA1 = inp("bA1T", [H, NT])
    t_bB1 = inp("bB1T", [H, NT])
    t_WA2 = inp("WA2T", [H, NT])
    t_WB2 = inp("WB2T", [H, NT])
    t_bA2 = inp("bA2_b", [128, NT])
    t_bB2 = inp("bB2_b", [128, NT])
    t_C = inp("C_b", [128, NT])
    t_Wi1 = inp("Wi1", [D, H])
    t_bi1 = inp("bi1_c", [H, 1])
    t_Wi2 = inp("Wi2_c", [H, 1])
    t_bi2 = inp("bi2_c", [1, 1])
    t_eye = inp("eye", [128, 128])

    t_out = nc.dram_tensor("out", [NT, 1], f32, kind="ExternalOutput")

    tvars = dict(locals())
    with tile.TileContext(nc) as tc:
        if LOOP_N > 1:
            with tc.For_i(0, LOOP_N, 1):
                _emit(nc, tc, tvars)
        else:
            _emit(nc, tc, tvars)
    nc.compile()
    return nc


def _emit(nc, tc, t):
    from contextlib import ExitStack
    ctx = ExitStack()
    with ctx:
        const = ctx.enter_context(tc.tile_pool(name="const", bufs=1))
        gsb = ctx.enter_context(tc.tile_pool(name="gsb", bufs=2))
        psb = ctx.enter_context(tc.tile_pool(name="psb", bufs=3))

        # ---------- load constants / inputs into SBUF ----------
        def load(name, shape, src_ap, pool=const):
            s = pool.tile(shape, f32, name=name)
            nc.sync.dma_start(out=s, in_=src_ap)
            return s

        Wemb = load("Wemb", [F_IN, D], t["t_Wemb"][:, :])
        h1T = load("h1T", [F_IN, N1], t["t_h1T"][:, :])
        h2T = load("h2T", [F_IN, N2], t["t_h2T"][:, :])
        eye = load("eye", [128, 128], t["t_eye"][:, :])
        gWb = load("gWb", [D, L], t["t_gWb"][:, :])
        gGb = load("gGb", [1, L], t["t_gGb"][:, :])
        Wi1 = load("Wi1", [D, H], t["t_Wi1"][:, :])
        bi1 = load("bi1", [H, 1], t["t_bi1"][:, :])
        Wi2 = load("Wi2", [H, 1], t["t_Wi2"][:, :])
        bi2 = load("bi2", [1, 1], t["t_bi2"][:, :])
        bA1 = load("bA1", [H, NT], t["t_bA1"][:, :])
        bB1 = load("bB1", [H, NT], t["t_bB1"][:, :])
        w2A = load("w2A", [H, NT], t["t_WA2"][:, :])
        w2B = load("w2B", [H, NT], t["t_WB2"][:, :])
        bA2 = load("bA2", [128, NT], t["t_bA2"][:, :])
        bB2 = load("bB2", [128, NT], t["t_bB2"][:, :])
        C_b = load("C_b", [128, NT], t["t_C"][:, :])
        valid = load("valid", [N1, 1], t["t_valid"][:, :])
        sum4 = load("sum4", [4 * NT, NT], t["t_sum4"][:, :])
        adj1T = load("adj1T", [N1, N1], t["t_adj1T"][:, :])
        dmv = const.tile([128, 4, N1 * 3], f32, name="dmv")
        for k in range(4):
            nc.sync.dma_start(out=dmv[:, k, :],
                              in_=t["t_dmv"][k * 128:(k + 1) * 128, :])

        gW = const.tile([D, L, D], f32, name="gW")
        gA = const.tile([D, L, D], f32, name="gA")
        gGW = const.tile([D, L, 2], f32, name="gGW")
        for l in range(L):
            nc.sync.dma_start(out=gW[:, l, :], in_=t["t_gW"][l, :, :])
            nc.sync.dma_start(out=gA[:, l, :], in_=t["t_gA"][l, :, :])
        nc.sync.dma_start(out=gGW, in_=t["t_gGW"][:, :, :])

        W1A = const.tile([D, NT, 2, H], f32, name="W1A")
        W1B = const.tile([D, NT, 2, H], f32, name="W1B")
        for ty in range(NT):
            for hf in range(2):
                nc.sync.dma_start(out=W1A[:, ty, hf, :], in_=t["t_WA1"][ty, hf, :, :])
                nc.sync.dma_start(out=W1B[:, ty, hf, :], in_=t["t_WB1"][ty, hf, :, :])

        adj2T = const.tile([128, 4, N2], f32, name="adj2T")
        for k in range(4):
            nc.sync.dma_start(out=adj2T[:, k, :],
                              in_=t["t_adj2T"][k * 128:(k + 1) * 128, :])
        Aint = const.tile([128, NT, 4, N1], f32, name="Aint")
        for ty in range(NT):
            for k in range(4):
                nc.sync.dma_start(out=Aint[:, ty, k, :],
                                  in_=t["t_Aint"][ty, k * 128:(k + 1) * 128, :])

        # derived constants
        mb2 = const.tile([128, 4, N2], f32, name="mb2")
        for k in range(4):
            nc.vector.tensor_scalar(mb2[:, k, :], adj2T[:, k, :], BIG, None,
                                    op0=AluOpType.mult)
        mb1 = const.tile([N1, N1], f32, name="mb1")
        nc.vector.tensor_scalar(mb1, adj1T, BIG, None, op0=AluOpType.mult)
        negC = const.tile([128, NT], f32, name="negC")
        nc.vector.tensor_scalar(negC, C_b, -1.0, None, op0=AluOpType.mult)
        halfgb = const.tile([1, L], f32, name="halfgb")
        nc.vector.tensor_scalar(halfgb, gGb, 0.5, None, op0=AluOpType.mult)
        ones64 = const.tile([N1, 1], f32, name="ones64")
        nc.vector.memset(ones64, 1.0)
        ones128 = const.tile([128, 1], f32, name="ones128")
        nc.vector.memset(ones128, 1.0)
        halfones = const.tile([1, 128], f32, name="halfones")
        nc.vector.memset(halfones, 0.5)
        c47 = const.tile([1, NT], f32, name="c47")
        nc.vector.memset(c47, 4.0 / NT)

        # ---------- dm = ||dmv|| (transposed [n2, n1] layout) ----------
        # Newton-refined sqrt: ACT sqrt alone is too inaccurate for the
        # dm < 0.5 sentinel threshold that dominates the output.
        dmsq = const.tile([128, 4, N1], f32, name="dmsq")
        dvsq = const.tile([128, N1 * 3], f32, name="dvsq")
        for k in range(4):
            nc.vector.tensor_mul(dvsq, dmv[:, k, :], dmv[:, k, :])
            nc.vector.reduce_sum(dmsq[:, k, :],
                                 dvsq.rearrange("p (n c) -> p n c", c=3),
                                 axis=AX.X)
        dmsq_f = dmsq.rearrange("p a b -> p (a b)")
        xp = const.tile([128, 4 * N1], f32, name="xp")
        nc.vector.tensor_scalar(xp, dmsq_f, 1e-10, None, op0=AluOpType.add)
        eps10 = const.tile([128, 1], f32, name="eps10")
        nc.vector.memset(eps10, 1e-10)
        s0 = const.tile([128, 4 * N1], f32, name="s0")
        nc.scalar.activation(s0, dmsq_f, AF.Sqrt, bias=eps10, scale=1.0)
        # two Newton iterations: s <- 0.5*(s + x/s)
        for it in range(2):
            r0 = const.tile([128, 4 * N1], f32, name=f"r{it}")
            nc.vector.reciprocal(r0, s0)
            m0 = const.tile([128, 4 * N1], f32, name=f"m{it}")
            nc.vector.tensor_mul(m0, xp, r0)
            s1 = const.tile([128, 4 * N1], f32, name=f"s{it + 1}")
            nc.vector.tensor_add(s1, s0, m0)
            nc.vector.tensor_scalar(s1, s1, 0.5, None, op0=AluOpType.mult)
            s0 = s1
        dm = const.tile([128, 4, N1], f32, name="dm")
        dm_f = dm.rearrange("p a b -> p (a b)")
        mflag = const.tile([128, 4 * N1], f32, name="mflag")
        nc.vector.tensor_scalar(mflag, s0, DM_MIN, None, op0=AluOpType.is_lt)
        nc.vector.scalar_tensor_tensor(dm_f, in0=mflag, scalar=1e10, in1=s0,
                                       op0=AluOpType.mult, op1=AluOpType.add)

        # ---------- embed ----------
        with tc.tile_pool(name="emb_ps", bufs=2, space="PSUM") as emb_ps:
            e1p = emb_ps.tile([D, N1], f32, tag="e", name="e1p")
            nc.tensor.matmul(e1p, lhsT=Wemb, rhs=h1T, start=True, stop=True)
            x1 = gsb.tile([D, N1], f32, tag="x1", name="x1_0")
            nc.scalar.copy(x1, e1p)
            e2p = emb_ps.tile([D, N2], f32, tag="e", name="e2p")
            nc.tensor.matmul(e2p, lhsT=Wemb, rhs=h2T, start=True, stop=True)
            x2 = gsb.tile([D, N2], f32, tag="x2", name="x2_0")
            nc.scalar.copy(x2, e2p)

        # ---------- GAT layers ----------
        def gat_layer(l, xT, N, CH, mb, sfx):
            nch = N // CH
            hTp = gps.tile([D, N], f32, tag="g" + sfx, name=f"hTp{sfx}{l}")
            nc.tensor.matmul(hTp, lhsT=gW[:, l, :], rhs=xT, start=True, stop=True)
            hT = gsb.tile([D, N], f32, tag="hT" + sfx, name=f"hT{sfx}{l}")
            nc.scalar.activation(hT, hTp, AF.Identity, bias=gWb[:, l:l + 1])
            uTp = gps.tile([D, N], f32, tag="g" + sfx, name=f"uTp{sfx}{l}")
            nc.tensor.matmul(uTp, lhsT=gA[:, l, :], rhs=hT, start=True, stop=True)
            uT = gsb.tile([D, N], f32, tag="uT" + sfx, name=f"uT{sfx}{l}")
            nc.scalar.copy(uT, uTp)
            hnat = gsb.tile([CH, nch, D], f32, tag="hn" + sfx, name=f"hn{sfx}{l}")
            for k in range(nch):
                tp = gps.tile([CH, D], f32, tag="g" + sfx, name=f"tp{sfx}{l}_{k}")
                nc.tensor.transpose(tp, hT[:, k * CH:(k + 1) * CH], eye)
                nc.scalar.copy(hnat[:, k, :], tp)
            Ta = gsb.tile([CH, nch, N], f32, tag="Ta" + sfx, name=f"Ta{sfx}{l}")
            for k in range(nch):
                ks = slice(k * CH, (k + 1) * CH)
                Fp = gps.tile([CH, N], f32, tag="g" + sfx, name=f"Fp{sfx}{l}_{k}")
                nc.tensor.matmul(Fp, lhsT=uT[:, ks], rhs=hT, start=True, stop=False)
                nc.tensor.matmul(Fp, lhsT=hT[:, ks], rhs=uT, start=False, stop=True)
                Fm = gsb.tile([CH, N], f32, tag="Fm" + sfx, name=f"Fm{sfx}{l}_{k}")
                nc.vector.tensor_add(Fm, Fp, mb[:, k, :] if nch > 1 else mb)
                nm = gsb.tile([CH, 1], f32, tag="nm" + sfx, name=f"nm{sfx}{l}_{k}")
                nc.vector.reduce_max(nm, Fm, axis=AX.X, negate=True)
                expF = gsb.tile([CH, N], f32, tag="ex" + sfx, name=f"ex{sfx}{l}_{k}")
                ssum = gsb.tile([CH, 1], f32, tag="ss" + sfx, name=f"ss{sfx}{l}_{k}")
                nc.scalar.activation(expF, Fm, AF.Exp, bias=nm, scale=1.0,
                                     accum_out=ssum)
                rs = gsb.tile([CH, 1], f32, tag="rs" + sfx, name=f"rs{sfx}{l}_{k}")
                nc.vector.reciprocal(rs, ssum)
                nc.vector.tensor_scalar(Ta[:, k, :], expF, rs, None,
                                        op0=AluOpType.mult)
            hpp = gps.tile([D, N], f32, tag="g" + sfx, name=f"hpp{sfx}{l}")
            for k in range(nch):
                nc.tensor.matmul(hpp, lhsT=hnat[:, k, :], rhs=Ta[:, k, :],
                                 start=(k == 0), stop=(k == nch - 1))
            hp = gsb.tile([D, N], f32, tag="hp" + sfx, name=f"hp{sfx}{l}")
            nc.scalar.activation(hp, hpp, AF.Relu)
            zp = gps.tile([1, N], f32, tag="g" + sfx, name=f"zp{sfx}{l}")
            nc.tensor.matmul(zp, lhsT=gGW[:, l, 0:1], rhs=xT, start=True, stop=False)
            nc.tensor.matmul(zp, lhsT=gGW[:, l, 1:2], rhs=hp, start=False, stop=True)
            cp = gsb.tile([1, N], f32, tag="cp" + sfx, name=f"cp{sfx}{l}")
            nc.scalar.activation(cp, zp, AF.Tanh, bias=halfgb[0:1, l:l + 1],
                                 scale=0.5)
            cbp = gps.tile([D, N], f32, tag="g" + sfx, name=f"cbp{sfx}{l}")
            nc.tensor.matmul(cbp, lhsT=halfones, rhs=cp, start=True, stop=True)
            d1 = gsb.tile([D, N], f32, tag="d1" + sfx, name=f"d1{sfx}{l}")
            nc.vector.tensor_sub(d1, xT, hp)
            t1 = gsb.tile([D, N], f32, tag="t1" + sfx, name=f"t1{sfx}{l}")
            nc.vector.scalar_tensor_tensor(t1, in0=d1, scalar=0.5, in1=hp,
                                           op0=AluOpType.mult, op1=AluOpType.add)
            t2 = gsb.tile([D, N], f32, tag="t2" + sfx, name=f"t2{sfx}{l}")
            nc.vector.tensor_mul(t2, d1, cbp)
            xn = gsb.tile([D, N], f32, tag="x" + sfx[0:1] + "n",
                          name=f"x{sfx}{l}n")
            nc.vector.tensor_add(xn, t1, t2)
            return xn

        if STAGE >= 2:
            with tc.tile_pool(name="gps_l", bufs=3, space="PSUM") as gps_l, \
                 tc.tile_pool(name="gps_p", bufs=4, space="PSUM") as gps_p:
                for l in range(L):
                    gps = gps_l
                    x1 = gat_layer(l, x1, N1, 64, mb1, "L")
                    gps = gps_p
                    x2 = gat_layer(l, x2, N2, 128, mb2, "P")

        h1eT, h2eT = x1, x2  # [D, N1], [D, N2]

        # ---------- pairwise interaction energies ----------
        # Layer 1 is rank-separable before the relu:
        #   hpair @ W1 = (h1e @ W1_top)[n1]  +  (h2e @ W1_bot)[n2]
        # Per (type, net, n1): X[h, n2] = relu(U1[:, n1] + U2)  (fused DVE
        # tensor_scalar add+max, or ACT bias-relu), then layer 2 is
        # arT[n2, n1] = X.T @ w2 via 4 stationary-X matmuls (N=1).  With
        # PAIR_BF16 the X path runs in bf16: DVE 4x mode + PE fast weight
        # load (2 bf16/cycle).
        dt_p = mybir.dt.bfloat16 if PAIR_BF16 else f32
        E28 = const.tile([128, NT, 4], f32, name="E28")
        n_act = int(round(ACT_RELU_FRAC * NT * 2 * N1))
        n_tot = NT * 2 * N1
        n_unit = 0

        if PAIR_BF16:
            w2A_c = const.tile([H, NT], dt_p, name="w2A_c")
            nc.vector.tensor_copy(w2A_c, w2A)
            w2B_c = const.tile([H, NT], dt_p, name="w2B_c")
            nc.vector.tensor_copy(w2B_c, w2B)
        else:
            w2A_c, w2B_c = w2A, w2B

        if STAGE < 3:
            nc.vector.memset(E28.rearrange("p a b -> p (a b)"), 0.0)
        with tc.tile_pool(name="u2ps", bufs=3, space="PSUM") as u2ps, \
             tc.tile_pool(name="arps", bufs=4, space="PSUM") as arps:
            for ty in range(NT if STAGE >= 3 else 0):
                U2sb, U1sb, U1sbf, w2c = [], [], [], []
                for net in range(2):
                    W1 = W1A if net == 0 else W1B
                    b1 = bA1 if net == 0 else bB1
                    w2 = w2A_c if net == 0 else w2B_c
                    u2p = u2ps.tile([H, N2], f32, tag="u2",
                                    name=f"u2p{ty}_{net}")
                    nc.tensor.matmul(u2p, lhsT=W1[:, ty, 1, :], rhs=h2eT,
                                     start=True, stop=True)
                    u2s = psb.tile([H, N2], dt_p, tag="u2s",
                                   name=f"u2s{ty}_{net}")
                    nc.scalar.copy(u2s, u2p)
                    u1p = u2ps.tile([H, N1], f32, tag="u2",
                                    name=f"u1p{ty}_{net}")
                    nc.tensor.matmul(u1p, lhsT=W1[:, ty, 0, :], rhs=h1eT,
                                     start=True, stop=True)
                    u1s = psb.tile([H, N1], f32, tag="u1s",
                                   name=f"u1s{ty}_{net}")
                    nc.scalar.activation(u1s, u1p, AF.Identity,
                                         bias=b1[:, ty:ty + 1])
                    U2sb.append(u2s)
                    U1sb.append(u1s)
                    U1sbf.append(u1s)
                    w2c.append(w2[:, ty:ty + 1])

                arT = []
                for net in range(2):
                    ar = arps.tile([128, 4, N1], f32, tag="ar",
                                   name=f"arT{ty}_{net}")
                    arT.append(ar)
                for n1 in range(N1):
                    for net in range(2):
                        X = psb.tile([H, N2], dt_p, tag="X",
                                     name=f"X{ty}_{n1}_{net}", bufs=8)
                        if (n_unit * n_act) % n_tot < n_act:
                            nc.scalar.activation(X, U2sb[net], AF.Relu,
                                                 bias=U1sbf[net][:, n1:n1 + 1],
                                                 scale=1.0)
                        else:
                            nc.vector.tensor_scalar(
                                X, U2sb[net], U1sb[net][:, n1:n1 + 1], 0.0,
                                op0=AluOpType.add, op1=AluOpType.max)
                        n_unit += 1
                        for k in range(4):
                            nc.tensor.matmul(
                                arT[net][:, k, n1:n1 + 1],
                                lhsT=X[:, k * 128:(k + 1) * 128],
                                rhs=w2c[net], start=True, stop=True)

                bc = float(BC_INV[ty])
                for k in range(4):
                    A_s = psb.tile([128, N1], f32, tag="As", name=f"As{ty}_{k}")
                    nc.scalar.activation(A_s, arT[0][:, k, :], AF.Sigmoid,
                                         bias=bA2[:, ty:ty + 1])
                    Bp_s = psb.tile([128, N1], f32, tag="Bs", name=f"Bs{ty}_{k}")
                    nc.scalar.activation(Bp_s, arT[1][:, k, :], AF.Sigmoid,
                                         bias=bB2[:, ty:ty + 1])
                    dsq = psb.tile([128, N1], f32, tag="dsq",
                                   name=f"dsq{ty}_{k}")
                    nc.scalar.activation(dsq, dm[:, k, :], AF.Square,
                                         bias=negC[:, ty:ty + 1])
                    # e = 4*(Bp*2bc*dsq + (bc*dsq - 1)) * A * A_int; the 4x
                    # is folded into the compile-time constants
                    kt = psb.tile([128, N1], f32, tag="kt", name=f"kt{ty}_{k}")
                    nc.vector.tensor_scalar(kt, dsq, 4.0 * bc, -4.0,
                                            op0=AluOpType.mult,
                                            op1=AluOpType.add)
                    t2e = psb.tile([128, N1], f32, tag="t2e",
                                   name=f"t2e{ty}_{k}")
                    nc.vector.scalar_tensor_tensor(t2e, in0=Bp_s,
                                                   scalar=8.0 * bc, in1=dsq,
                                                   op0=AluOpType.mult,
                                                   op1=AluOpType.mult)
                    t3e = psb.tile([128, N1], f32, tag="t3e",
                                   name=f"t3e{ty}_{k}")
                    nc.vector.tensor_add(t3e, t2e, kt)
                    t4e = psb.tile([128, N1], f32, tag="t4e",
                                   name=f"t4e{ty}_{k}")
                    nc.vector.tensor_mul(t4e, t3e, A_s)
                    t5e = psb.tile([128, N1], f32, tag="t5e",
                                   name=f"t5e{ty}_{k}")
                    nc.vector.tensor_mul(t5e, t4e, Aint[:, ty, k, :])
                    nc.vector.reduce_sum(E28[:, ty, k:k + 1], t5e, axis=AX.X)

        # ---------- intercept + final reduce ----------
        with tc.tile_pool(name="fin_ps", bufs=3, space="PSUM") as fin_ps:
            h1p = fin_ps.tile([N1, D], f32, tag="f", name="h1p")
            nc.tensor.transpose(h1p, h1eT, eye)
            h1n = psb.tile([N1, D], f32, tag="h1n", name="h1n")
            nc.scalar.copy(h1n, h1p)
            hm = psb.tile([N1, D], f32, tag="hm", name="hm")
            nc.vector.tensor_scalar(hm, h1n, valid[:, 0:1], None,
                                    op0=AluOpType.mult)
            poolp = fin_ps.tile([D, 1], f32, tag="f", name="poolp")
            nc.tensor.matmul(poolp, lhsT=hm, rhs=ones64, start=True, stop=True)
            pooled = psb.tile([D, 1], f32, tag="pooled", name="pooled")
            nc.scalar.copy(pooled, poolp)
            z1p = fin_ps.tile([H, 1], f32, tag="f", name="z1p")
            nc.tensor.matmul(z1p, lhsT=Wi1, rhs=pooled, start=True, stop=True)
            r1 = psb.tile([H, 1], f32, tag="r1", name="r1")
            nc.scalar.activation(r1, z1p, AF.Relu, bias=bi1)
            z2p = fin_ps.tile([1, 1], f32, tag="f", name="z2p")
            nc.tensor.matmul(z2p, lhsT=Wi2, rhs=r1, start=True, stop=True)
            icpt = psb.tile([1, 1], f32, tag="icpt", name="icpt")
            nc.scalar.activation(icpt, z2p, AF.Sigmoid, bias=bi2[0:1, 0:1])
            # sum E28 over its 128 partitions, then over the 4 n2-chunks,
            # then add intercept*(4/7)
            Ep28 = fin_ps.tile([4 * NT, 1], f32, tag="f", name="Ep28")
            nc.tensor.matmul(Ep28, lhsT=E28.rearrange("p a b -> p (a b)"),
                             rhs=ones128, start=True, stop=True)
            E28s = psb.tile([4 * NT, 1], f32, tag="E28s", name="E28s")
            nc.scalar.copy(E28s, Ep28)
            Ep = fin_ps.tile([NT, 1], f32, tag="f", name="Ep")
            nc.tensor.matmul(Ep, lhsT=sum4, rhs=E28s, start=True, stop=False)
            nc.tensor.matmul(Ep, lhsT=c47, rhs=icpt, start=False, stop=True)
            outs = psb.tile([NT, 1], f32, tag="outs", name="outs")
            nc.scalar.copy(outs, Ep)
            nc.sync.dma_start(out=t["t_out"][:, :], in_=outs)


def _in_maps(inputs):
    f = np.float32
    c = np.ascontiguousarray
    h1, h2 = inputs["h1"], inputs["h2"]
    adj1, adj2 = inputs["adj1"], inputs["adj2"]
    A_int, dmv, valid = inputs["A_int"], inputs["dmv"], inputs["valid"]
    WA1 = inputs["WA1"].reshape(NT, 2, D, H)
    WB1 = inputs["WB1"].reshape(NT, 2, D, H)
    shared = {
        "W_embed": c(inputs["W_embed"], dtype=f),
        "gW": c(inputs["gW"], dtype=f),
        "gA": c(inputs["gA"], dtype=f),
        "gWbT": c(inputs["gWb"].T, dtype=f),
        "gGateW_s": c(inputs["gGateW"].reshape(L, 2, D).transpose(2, 0, 1), dtype=f),
        "gGateb_r": c(inputs["gGateb"].reshape(1, L), dtype=f),
        "WA1_s": c(WA1, dtype=f),
        "WB1_s": c(WB1, dtype=f),
        "bA1T": c(inputs["bA1"].T, dtype=f),
        "bB1T": c(inputs["bB1"].T, dtype=f),
        "WA2T": c(inputs["WA2"].T, dtype=f),
        "WB2T": c(inputs["WB2"].T, dtype=f),
        "bA2_b": c(np.broadcast_to(inputs["bA2"].reshape(1, NT), (128, NT)), dtype=f),
        "bB2_b": c(np.broadcast_to(inputs["bB2"].reshape(1, NT), (128, NT)), dtype=f),
        "C_b": c(np.broadcast_to(inputs["C"].reshape(1, NT), (128, NT)), dtype=f),
        "sum4": np.repeat(np.eye(NT, dtype=f), 4, axis=0),
        "Wi1": c(inputs["Wi1"], dtype=f),
        "bi1_c": c(inputs["bi1"].reshape(H, 1), dtype=f),
        "Wi2_c": c(inputs["Wi2"].reshape(H, 1), dtype=f),
        "bi2_c": c(inputs["bi2"].reshape(1, 1), dtype=f),
        "eye": np.eye(128, dtype=f),
    }
    maps = []
    for b in range(B):
        m = dict(shared)
        m["h1T"] = c(h1[b].T, dtype=f)
        m["h2T"] = c(h2[b].T, dtype=f)
        m["adj1T"] = c(adj1[b].T, dtype=f)
        m["adj2T"] = c(adj2[b].T, dtype=f)
        m["A_intT"] = c(A_int[b].transpose(0, 2, 1), dtype=f)
        m["dmvT"] = c(dmv[b].transpose(1, 0, 2).reshape(N2, N1 * 3), dtype=f)
        m["valid"] = c(valid[b].reshape(N1, 1), dtype=f)
        maps.append(m)
    return maps


def _make_runner(nc, n_cores):
    """Persistent jitted SPMD runner (mirrors bass2jax.run_bass_via_pjrt but
    caches the compiled executable so repeat calls don't re-lower)."""
    import jax
    import concourse.mybir as mybir_
    from concourse import bass2jax
    from jax.experimental.shard_map import shard_map
    from jax.sharding import Mesh, PartitionSpec

    bass2jax.install_neuronx_cc_hook()
    partition_name = nc.partition_id_tensor.name if nc.partition_id_tensor else None
    in_names, out_names, out_avals, zero_outs = [], [], [], []
    for alloc in nc.m.functions[0].allocations:
        if not isinstance(alloc, mybir_.MemoryLocationSet):
            continue
        name = alloc.memorylocations[0].name
        if alloc.kind == "ExternalInput":
            if name != partition_name:
                in_names.append(name)
        elif alloc.kind == "ExternalOutput":
            shape = tuple(alloc.tensor_shape)
            dtype = mybir_.dt.np(alloc.dtype)
            out_names.append(name)
            out_avals.append(jax.core.ShapedArray(shape, dtype))
            zero_outs.append(np.zeros(shape, dtype))
    n_params = len(in_names)
    n_outs = len(out_avals)
    all_in = list(in_names) + list(out_names)
    if partition_name is not None:
        all_in.append(partition_name)
    donate = tuple(range(n_params, n_params + n_outs))

    def _body(*args):
        operands = list(args)
        if partition_name is not None:
            operands.append(bass2jax.partition_id_tensor())
        outs = bass2jax._bass_exec_p.bind(
            *operands,
            out_avals=tuple(out_avals),
            in_names=tuple(all_in),
            out_names=tuple(out_names),
            lowering_input_output_aliases=(),
            sim_require_finite=True,
            sim_require_nnan=True,
            nc=nc,
        )
        return tuple(outs)

    devices = jax.devices()[:n_cores]
    mesh = Mesh(np.asarray(devices), ("core",))
    sharded = jax.jit(
        shard_map(_body, mesh=mesh,
                  in_specs=(PartitionSpec("core"),) * (n_params + n_outs),
                  out_specs=(PartitionSpec("core"),) * n_outs,
                  check_rep=False),
        donate_argnums=donate, keep_unused=True)

    def run(in_maps, timing_reps=0):
        concat_in = [
            np.concatenate([np.asarray(m[name]) for m in in_maps], axis=0)
            for name in in_names
        ]
        concat_zeros = [
            np.zeros((n_cores * z.shape[0], *z.shape[1:]), z.dtype)
            for z in zero_outs
        ]
        out_arrs = sharded(*concat_in, *concat_zeros)
        out_arrs = [np.asarray(a) for a in out_arrs]
        if timing_reps:
            import time
            from jax.sharding import NamedSharding
            shard = NamedSharding(mesh, PartitionSpec("core"))
            dev_in = [jax.device_put(x, shard) for x in concat_in]
            jax.block_until_ready(dev_in)

            def one():
                zs = [np.zeros((n_cores * z.shape[0], *z.shape[1:]), z.dtype)
                      for z in zero_outs]
                return sharded(*dev_in, *zs)

            jax.block_until_ready(one())
            times = []
            for _ in range(timing_reps):
                t0 = time.perf_counter()
                r = one()
                jax.block_until_ready(r)
                times.append(time.perf_counter() - t0)
            times.sort()
            LAST_RESULT["wall_per_call_s"] = times[0]
            LAST_RESULT["wall_median_s"] = times[len(times) // 2]
            LAST_RESULT["wall_all"] = times
        return [
            {name: out_arrs[i].reshape(n_cores, *out_avals[i].shape)[c]
             for i, name in enumerate(out_names)}
            for c in range(n_cores)
        ]

    return run


def kernel(**inputs):
    inputs = {k: np.asarray(v) for k, v in inputs.items()}
    if "nc" not in _cache:
        _cache["nc"] = _build()
        _cache["run"] = _make_runner(_cache["nc"], B)
    in_maps = _in_maps(inputs)
    results = _cache["run"](in_maps, timing_reps=TIMING_REPS)
    out = np.stack([results[b]["out"][:, 0] for b in range(B)], axis=0)
    return out.astype(np.float32)



# revision 38
# speedup vs baseline: 1.6624x; 1.6624x over previous
# Trainium2 Bass kernel for nn_DTIHarmonicIS (DTI_PDBbind-style GAT + pairwise
# harmonic interaction energies). Data-parallel over batch B=8 across 8 cores.
#
# Self-contained: hardcodes all shapes/sharding. kernel(**inputs) takes FULL
# inputs (as produced by setup_inputs) and returns the FULL [B, 7] output.
#
# Layout/engine strategy:
#  - bf16 datapath for all large matmuls (PE 1 cycle/row vs 4 for fp32).
#  - GAT softmax uses negative masking ((adj-1)*1000) so no reduce_max pass
#    is needed: masked logits underflow to exact 0 in exp, live logits are
#    O(10) so exp cannot overflow.
#  - dm: sentinel flag computed exactly from dmsq (< 0.25), single ACT sqrt,
#    no Newton refinement.
#  - pairwise relu units relu(U1[:,n1] + U2) are split across DVE (bf16 4x
#    mode), ACT (reads PSUM directly), and GPSIMD/Pool.
#  - inputs are host-packed into a few wide DMAs, issued in dependency order.

import os
import numpy as np

import concourse.bass as bass
import concourse.bacc as bacc
import concourse.tile as tile
import concourse.mybir as mybir
from concourse.alu_op_type import AluOpType
from concourse.bass_utils import run_bass_kernel_spmd

B, N1, N2, D, L, H, NT = 8, 64, 512, 128, 3, 128, 7
F_IN = 56
BIG = 1000.0  # negative mask offset; exp(-1000+x) underflows to exact 0
B_CONSTRAINT = np.array([1.159, 0.448, 0.927, 0.902, 0.349, 0.789, 0.198],
                        np.float32)
BC_INV = (1.0 / (3.0 * B_CONSTRAINT ** 2)).astype(np.float32)

f32 = mybir.dt.float32
bf16 = mybir.dt.bfloat16
AF = mybir.ActivationFunctionType
AX = mybir.AxisListType

# X-relu engine split, per 32 units: KXA on ACT, KXP on Pool, rest on DVE.
KXA = int(os.environ.get('KXA', '5'))
KXP = int(os.environ.get('KXP', '9'))
KCOMB = os.environ.get('KCOMB', 'dve')  # 'pool' offloads combine mul chain

LOOP_N = int(os.environ.get('KLOOP', '1'))  # >1: repeat body in-NEFF (timing)
TIMING_REPS = 0         # set >0 (e.g. from test.py) to wall-clock repeat runs
LAST_RESULT = {}        # timing info stashed here after each run

_cache = {}

# cpack (fp32, [128, C]) column map
C_BA1, C_BB1, C_NEGC, C_BA2, C_BB2H = 0, 7, 14, 21, 28
C_WI1, C_BI1, C_WI2, C_BI2, C_EPS = 35, 163, 164, 165, 166
C_GWB = 167
C_UB = 170
C_TOT = 173

# gatpk (bf16, [128, G]) column map: gW(384) gAW(384) gGW(6) eye(128) eyeK(128)
# gAW[l] = gW[l] @ gA[l] host-folded so uT doesn't wait on hT
# eyeK = 1000*eye: folds the softmax mask into the Fp PSUM accumulation
G_GW, G_GAW, G_GGW, G_EYE, G_EYEK = 0, 384, 768, 774, 902
G_TOT = 1030

# w1pk (bf16, [128, W]): W1A(1792) W1B(1792) w2A(7) w2B(7)
W_A, W_B, W_W2A, W_W2B, W_TOT = 0, 1792, 3584, 3591, 3598


def _build():
    nc = bacc.Bacc("TRN2", target_bir_lowering=False)

    def inp(name, shape, dt=f32):
        return nc.dram_tensor(name, shape, dt, kind="ExternalInput")

    t = {
        # per-core (batch-sliced) data, in DMA priority order
        "cpack": inp("cpack", [128, C_TOT]),
        "pk56": inp("pk56", [F_IN, N1 + N2 + D], bf16),
        "gatpk": inp("gatpk", [128, G_TOT], bf16),
        "dmvT": inp("dmvT", [128, 4 * N1 * 3]),
        "mb2": inp("mb2", [128, 4 * N2], bf16),
        "mb1": inp("mb1", [N1, N1], bf16),
        "p1": inp("p1", [1, 10]),
        "w1pk": inp("w1pk", [128, W_TOT], bf16),
        "aint": inp("aint", [128, NT * 4 * N1], bf16),
        "valid": inp("valid", [N1, 1]),
        "sum4x": inp("sum4x", [4 * NT, NT]),
    }
    t["out"] = nc.dram_tensor("out", [NT, 1], f32, kind="ExternalOutput")

    with tile.TileContext(nc) as tc:
        if LOOP_N > 1:
            with tc.For_i(0, LOOP_N, 1):
                _emit(nc, tc, t)
        else:
            _emit(nc, tc, t)
    nc.compile()
    return nc


def _emit(nc, tc, t):
    from contextlib import ExitStack
    ctx = ExitStack()
    with ctx:
        const = ctx.enter_context(tc.tile_pool(name="const", bufs=1))
        gsb = ctx.enter_context(tc.tile_pool(name="gsb", bufs=2))
        psb = ctx.enter_context(tc.tile_pool(name="psb", bufs=3))

        def load(name, shape, src_ap, dt=f32):
            s = const.tile(shape, dt, name=name)
            nc.sync.dma_start(out=s, in_=src_ap)
            return s

        # ---------- input DMAs (priority order) ----------
        cpack = load("cpack", [128, C_TOT], t["cpack"][:, :])
        pk56 = load("pk56", [F_IN, N1 + N2 + D], t["pk56"][:, :], bf16)
        gatpk = load("gatpk", [128, G_TOT], t["gatpk"][:, :], bf16)
        dmv = load("dmv", [128, 4, N1 * 3], t["dmvT"][:, :])
        mb2 = load("mb2", [128, 4, N2], t["mb2"][:, :], bf16)
        mb1 = load("mb1", [N1, N1], t["mb1"][:, :], bf16)
        p1 = load("p1", [1, 10], t["p1"][:, :])
        w1pk = load("w1pk", [128, W_TOT], t["w1pk"][:, :], bf16)
        aint = load("aint", [128, NT, 4, N1], t["aint"][:, :], bf16)
        valid = load("valid", [N1, 1], t["valid"][:, :])
        sum4x = load("sum4x", [4 * NT, NT], t["sum4x"][:, :])

        h1T = pk56[:, 0:N1]
        h2T = pk56[:, N1:N1 + N2]
        Wemb = pk56[:, N1 + N2:]
        eye = gatpk[:, G_EYE:G_EYE + 128]
        eyeK = gatpk[:, G_EYEK:G_EYEK + 128]

        def gW(l):
            return gatpk[:, G_GW + l * D:G_GW + (l + 1) * D]

        def gAW(l):
            return gatpk[:, G_GAW + l * D:G_GAW + (l + 1) * D]

        def gGW(l, s):
            return gatpk[:, G_GGW + 2 * l + s:G_GGW + 2 * l + s + 1]

        def cp_col(base, ty=0, w=1):
            return cpack[:, base + ty:base + ty + w]

        # memset-constants
        ones64 = const.tile([N1, 1], bf16, name="ones64")
        nc.vector.memset(ones64, 1.0)
        ones128 = const.tile([128, 1], f32, name="ones128")
        nc.vector.memset(ones128, 1.0)
        halfones = const.tile([1, 128], bf16, name="halfones")
        nc.vector.memset(halfones, 0.5)
        onesrow = const.tile([1, N2], bf16, name="onesrow")
        nc.vector.memset(onesrow, 1.0)
        c47 = const.tile([1, NT], f32, name="c47")
        nc.vector.memset(c47, 4.0 / NT)

        # Prime the ACT table with the sqrt set before any other ACT work so
        # the static table-load sequence is sqrt -> exp(GAT) -> sigmoid(pair).
        prime = const.tile([1, 1], f32, name="prime")
        nc.scalar.activation(prime, cpack[0:1, C_EPS:C_EPS + 1], AF.Sqrt)

        # ---------- dm (transposed [n2, n1] layout, [128, 4, N1]) ----------
        dmsq = const.tile([128, 4, N1], f32, name="dmsq")
        for k in range(4):
            dvsq = psb.tile([128, N1 * 3], f32, tag="dvsq", name=f"dvsq{k}")
            nc.vector.tensor_mul(dvsq, dmv[:, k, :], dmv[:, k, :])
            nc.vector.reduce_sum(dmsq[:, k, :],
                                 dvsq.rearrange("p (n c) -> p n c", c=3),
                                 axis=AX.X)
        dmsq_f = dmsq.rearrange("p a b -> p (a b)")
        mflag = const.tile([128, 4 * N1], f32, name="mflag")
        nc.vector.tensor_scalar(mflag, dmsq_f, 0.25, None, op0=AluOpType.is_lt)
        sq = const.tile([128, 4 * N1], f32, name="sq")
        nc.scalar.activation(sq, dmsq_f, AF.Sqrt, bias=cp_col(C_EPS))
        dm = const.tile([128, 4, N1], f32, name="dm")
        nc.vector.scalar_tensor_tensor(dm.rearrange("p a b -> p (a b)"),
                                       in0=mflag, scalar=1e10, in1=sq,
                                       op0=AluOpType.mult, op1=AluOpType.add)


        # ---------- embed ----------
        with tc.tile_pool(name="emb_ps", bufs=2, space="PSUM") as emb_ps:
            e1p = emb_ps.tile([D, N1], f32, tag="e", name="e1p")
            nc.tensor.matmul(e1p, lhsT=Wemb, rhs=h1T, start=True, stop=True)
            x1 = gsb.tile([D, N1], bf16, tag="x1", name="x1_0")
            nc.scalar.copy(x1, e1p)
            e2p = emb_ps.tile([D, N2], f32, tag="e", name="e2p")
            nc.tensor.matmul(e2p, lhsT=Wemb, rhs=h2T, start=True, stop=True)
            x2 = gsb.tile([D, N2], bf16, tag="x2", name="x2_0")
            nc.scalar.copy(x2, e2p)

        # ---------- GAT layers (bf16 matmul path) ----------
        def gat_layer(l, xT, N, CH, mb, sfx):
            nch = N // CH
            # ligand graph is small; push its SBUF-only vector ops to the
            # otherwise-idle Pool engine
            ve = nc.gpsimd if sfx == "L" else nc.vector
            hTp = gps.tile([D, N], f32, tag="g" + sfx, name=f"hTp{sfx}{l}")
            nc.tensor.matmul(hTp, lhsT=gW(l), rhs=xT, start=True, stop=True)
            hT = gsb.tile([D, N], bf16, tag="hT" + sfx, name=f"hT{sfx}{l}")
            nc.scalar.activation(hT, hTp, AF.Identity,
                                 bias=cp_col(C_GWB, l))
            uTp = gps.tile([D, N], f32, tag="g" + sfx, name=f"uTp{sfx}{l}")
            nc.tensor.matmul(uTp, lhsT=gAW(l), rhs=xT, start=True, stop=True)
            uT = gsb.tile([D, N], bf16, tag="uT" + sfx, name=f"uT{sfx}{l}")
            nc.scalar.activation(uT, uTp, AF.Identity, bias=cp_col(C_UB, l))
            hnat = gsb.tile([CH, nch, D], bf16, tag="hn" + sfx,
                            name=f"hn{sfx}{l}")
            for k in range(nch):
                tp = gps.tile([CH, D], bf16, tag="g" + sfx,
                              name=f"tp{sfx}{l}_{k}")
                nc.tensor.transpose(tp, hT[:, k * CH:(k + 1) * CH], eye)
                nc.vector.tensor_copy(hnat[:, k, :], tp)
            Ta = gsb.tile([CH, nch, N], bf16, tag="Ta" + sfx,
                          name=f"Ta{sfx}{l}")
            for k in range(nch):
                ks = slice(k * CH, (k + 1) * CH)
                Fp = gps.tile([CH, N], f32, tag="g" + sfx,
                              name=f"Fp{sfx}{l}_{k}")
                nc.tensor.matmul(Fp, lhsT=uT[:, ks], rhs=hT,
                                 start=True, stop=False)
                nc.tensor.matmul(Fp, lhsT=hT[:, ks], rhs=uT,
                                 start=False, stop=False)
                nc.tensor.matmul(Fp, lhsT=eyeK[0:CH, 0:CH],
                                 rhs=mb[:, k, :] if nch > 1 else mb,
                                 start=False, stop=True)
                expF = gsb.tile([CH, N], bf16, tag="ex" + sfx,
                                name=f"ex{sfx}{l}_{k}")
                ssum = gsb.tile([CH, 1], f32, tag="ss" + sfx,
                                name=f"ss{sfx}{l}_{k}")
                nc.scalar.activation(expF, Fp, AF.Exp, accum_out=ssum)
                rs = gsb.tile([CH, 1], f32, tag="rs" + sfx,
                              name=f"rs{sfx}{l}_{k}")
                nc.vector.reciprocal(rs, ssum)
                ta_eng = nc.gpsimd if (sfx == "L" or k % 2) else nc.vector
                ta_eng.tensor_scalar(Ta[:, k, :], expF, rs, None,
                                     op0=AluOpType.mult)
            hpp = gps.tile([D, N], f32, tag="g" + sfx, name=f"hpp{sfx}{l}")
            for k in range(nch):
                nc.tensor.matmul(hpp, lhsT=hnat[:, k, :], rhs=Ta[:, k, :],
                                 start=(k == 0), stop=(k == nch - 1))
            hp = gsb.tile([D, N], bf16, tag="hp" + sfx, name=f"hp{sfx}{l}")
            nc.scalar.activation(hp, hpp, AF.Relu)
            zp = gps.tile([1, N], f32, tag="g" + sfx, name=f"zp{sfx}{l}")
            nc.tensor.matmul(zp, lhsT=gGW(l, 0), rhs=xT, start=True,
                             stop=False)
            nc.tensor.matmul(zp, lhsT=gGW(l, 1), rhs=hp, start=False,
                             stop=True)
            cp = gsb.tile([1, N], bf16, tag="cp" + sfx, name=f"cp{sfx}{l}")
            nc.scalar.activation(cp, zp, AF.Tanh, bias=p1[0:1, l:l + 1],
                                 scale=0.5)
            # cbp = broadcast of the full gate c = 0.5 + 0.5*tanh(...), via
            # two accumulated rank-1 matmuls; then xn = hp + c*(x - hp)
            cbp = gps.tile([D, N], f32, tag="g" + sfx, name=f"cbp{sfx}{l}")
            nc.tensor.matmul(cbp, lhsT=halfones, rhs=cp, start=True,
                             stop=False)
            nc.tensor.matmul(cbp, lhsT=halfones, rhs=onesrow[:, 0:N],
                             start=False, stop=True)
            d1 = gsb.tile([D, N], bf16, tag="d1" + sfx, name=f"d1{sfx}{l}")
            ve.tensor_sub(d1, xT, hp)
            t2 = gsb.tile([D, N], bf16, tag="t2" + sfx, name=f"t2{sfx}{l}")
            nc.vector.tensor_mul(t2, d1, cbp)
            xn = gsb.tile([D, N], bf16, tag="x" + sfx[0:1] + "n",
                          name=f"x{sfx}{l}n")
            ve.tensor_add(xn, t2, hp)
            return xn

        with tc.tile_pool(name="gps_l", bufs=3, space="PSUM") as gps_l, \
             tc.tile_pool(name="gps_p", bufs=4, space="PSUM") as gps_p:
            for l in range(L):
                gps = gps_l
                x1 = gat_layer(l, x1, N1, 64, mb1, "L")
                gps = gps_p
                x2 = gat_layer(l, x2, N2, 128, mb2, "P")

        h1eT, h2eT = x1, x2  # bf16 [D, N1], [D, N2]

        # ---------- intercept MLP (independent of pairwise; runs early) ----
        with tc.tile_pool(name="ic_ps", bufs=2, space="PSUM") as ic_ps:
            h1p = ic_ps.tile([N1, D], bf16, tag="f", name="h1p")
            nc.tensor.transpose(h1p, h1eT, eye)
            hm = psb.tile([N1, D], bf16, tag="hm", name="hm")
            nc.vector.tensor_scalar(hm, h1p, valid[:, 0:1], None,
                                    op0=AluOpType.mult)
            poolp = ic_ps.tile([D, 1], f32, tag="f", name="poolp")
            nc.tensor.matmul(poolp, lhsT=hm, rhs=ones64, start=True,
                             stop=True)
            pooled = psb.tile([D, 1], f32, tag="pooled", name="pooled")
            nc.scalar.copy(pooled, poolp)
            z1p = ic_ps.tile([H, 1], f32, tag="f", name="z1p")
            nc.tensor.matmul(z1p, lhsT=cpack[:, C_WI1:C_WI1 + 128],
                             rhs=pooled, start=True, stop=True)
            r1 = psb.tile([H, 1], f32, tag="r1", name="r1")
            nc.scalar.activation(r1, z1p, AF.Relu, bias=cp_col(C_BI1))
            z2p = ic_ps.tile([1, 1], f32, tag="f", name="z2p")
            nc.tensor.matmul(z2p, lhsT=cp_col(C_WI2), rhs=r1, start=True,
                             stop=True)
            # sigmoid via exp (keeps the exp ACT-table resident; a lone
            # Sigmoid here would cost two extra table reloads)
            en = psb.tile([1, 1], f32, tag="icpt", name="en")
            nc.scalar.activation(en, z2p, AF.Exp, scale=-1.0,
                                 bias=cpack[0:1, C_BI2:C_BI2 + 1])
            ep1 = psb.tile([1, 1], f32, tag="icpt", name="ep1")
            nc.vector.tensor_scalar(ep1, en, 1.0, None, op0=AluOpType.add)
            icpt = psb.tile([1, 1], f32, tag="icpt", name="icpt")
            nc.vector.reciprocal(icpt, ep1)

        # ---------- pairwise interaction energies ----------
        E28 = const.tile([128, NT, 4], f32, name="E28")
        # engine pattern for the 2*N1 relu units per type
        pat = []
        for i in range(32):
            if i % 32 < KXA:
                pat.append('a')
            elif i % 32 < KXA + KXP:
                pat.append('p')
            else:
                pat.append('d')
        unit_idx = 0

        comb_eng = nc.gpsimd if KCOMB == 'pool' else nc.vector

        with tc.tile_pool(name="u2ps", bufs=3, space="PSUM") as u2ps, \
             tc.tile_pool(name="arps", bufs=4, space="PSUM") as arps:

            def prep(ty):
                """U1/U2 matmuls + copies + dsq for type ty (pipelined one
                type ahead of the relu-unit loop)."""
                u2s, u1s, w2c = [], [], []
                for net in range(2):
                    wbase = W_A if net == 0 else W_B
                    w2b = W_W2A if net == 0 else W_W2B
                    bcol = C_BA1 if net == 0 else C_BB1
                    w1 = slice(wbase + (ty * 2 + 1) * H,
                               wbase + (ty * 2 + 2) * H)
                    u2pt = u2ps.tile([H, N2], f32, tag="u2",
                                     name=f"u2p{ty}_{net}")
                    nc.tensor.matmul(u2pt, lhsT=w1pk[:, w1], rhs=h2eT,
                                     start=True, stop=True)
                    u2st = psb.tile([H, N2], bf16, tag="u2s",
                                    name=f"u2s{ty}_{net}", bufs=4)
                    nc.scalar.copy(u2st, u2pt)
                    w0 = slice(wbase + (ty * 2) * H, wbase + (ty * 2 + 1) * H)
                    u1p = u2ps.tile([H, N1], f32, tag="u2",
                                    name=f"u1p{ty}_{net}")
                    nc.tensor.matmul(u1p, lhsT=w1pk[:, w0], rhs=h1eT,
                                     start=True, stop=True)
                    u1st = psb.tile([H, N1], f32, tag="u1s",
                                    name=f"u1s{ty}_{net}", bufs=4)
                    nc.vector.tensor_scalar(u1st, u1p,
                                            cp_col(bcol, ty), None,
                                            op0=AluOpType.add)
                    u2s.append(u2st)
                    u1s.append(u1st)
                    w2c.append(w1pk[:, w2b + ty:w2b + ty + 1])
                dsq = psb.tile([128, 4 * N1], f32, tag="dsq",
                               name=f"dsq{ty}", bufs=3)
                nc.scalar.activation(dsq, dm.rearrange("p a b -> p (a b)"),
                                     AF.Square, bias=cp_col(C_NEGC, ty))
                return u2s, u1s, w2c, dsq

            pr = prep(0)
            for ty in range(NT):
                u2s, u1s, w2c, dsq = pr
                if ty + 1 < NT:
                    pr = prep(ty + 1)

                arT = []
                for net in range(2):
                    ar = arps.tile([128, 4, N1], f32, tag="ar",
                                   name=f"arT{ty}_{net}")
                    arT.append(ar)
                for n1 in range(N1):
                    for net in range(2):
                        eng = pat[unit_idx % 32]
                        unit_idx += 1
                        X = psb.tile([H, N2], bf16, tag="X" + eng,
                                     name=f"X{ty}_{n1}_{net}",
                                     bufs=16 if eng == 'd' else 6)
                        if eng == 'a':
                            nc.scalar.activation(X, u2s[net], AF.Relu,
                                                 bias=u1s[net][:, n1:n1 + 1])
                        elif eng == 'p':
                            nc.gpsimd.tensor_scalar(
                                X, u2s[net], u1s[net][:, n1:n1 + 1], 0.0,
                                op0=AluOpType.add, op1=AluOpType.max)
                        else:
                            nc.vector.tensor_scalar(
                                X, u2s[net], u1s[net][:, n1:n1 + 1], 0.0,
                                op0=AluOpType.add, op1=AluOpType.max)
                        for k in range(4):
                            nc.tensor.matmul(
                                arT[net][:, k, n1:n1 + 1],
                                lhsT=X[:, k * 128:(k + 1) * 128],
                                rhs=w2c[net], start=True, stop=True)

                # combine: e(ty) = 4*sigma(arA+bA2) *
                #   (bc*(2+tanh((arB+bB2)/2))*(dm-C)^2 - 1) * A_int
                # (the 4x is folded into sum4x host-side)
                bc = float(BC_INV[ty])
                arA = arT[0].rearrange("p a b -> p (a b)")
                arB = arT[1].rearrange("p a b -> p (a b)")
                A_s = psb.tile([128, 4 * N1], f32, tag="As", name=f"As{ty}")
                nc.scalar.activation(A_s, arA, AF.Sigmoid,
                                     bias=cp_col(C_BA2, ty))
                T_s = psb.tile([128, 4 * N1], f32, tag="Ts", name=f"Ts{ty}")
                nc.scalar.activation(T_s, arB, AF.Tanh,
                                     bias=cp_col(C_BB2H, ty), scale=0.5)
                tmp = psb.tile([128, 4 * N1], f32, tag="tmp", name=f"tmp{ty}")
                comb_eng.scalar_tensor_tensor(tmp, in0=T_s, scalar=2.0,
                                              in1=dsq, op0=AluOpType.add,
                                              op1=AluOpType.mult)
                u = psb.tile([128, 4 * N1], f32, tag="u", name=f"u{ty}")
                comb_eng.tensor_scalar(u, tmp, bc, -1.0,
                                       op0=AluOpType.mult, op1=AluOpType.add)
                v = psb.tile([128, 4 * N1], f32, tag="v", name=f"v{ty}")
                nc.vector.tensor_mul(v, u, A_s)
                w = psb.tile([128, 4, N1], f32, tag="w", name=f"w{ty}")
                nc.vector.tensor_mul(w.rearrange("p a b -> p (a b)"), v,
                                     aint[:, ty, :, :].rearrange(
                                         "p a b -> p (a b)"))
                nc.vector.reduce_sum(E28[:, ty, :], w, axis=AX.X)

        # ---------- final reduce ----------
        with tc.tile_pool(name="fin_ps", bufs=2, space="PSUM") as fin_ps:
            # sum E28 over partitions, then over the 4 n2-chunks (x4 via
            # sum4x), then add intercept*(4/7)
            Ep28 = fin_ps.tile([4 * NT, 1], f32, tag="f", name="Ep28")
            nc.tensor.matmul(Ep28, lhsT=E28.rearrange("p a b -> p (a b)"),
                             rhs=ones128, start=True, stop=True)
            E28s = psb.tile([4 * NT, 1], f32, tag="E28s", name="E28s")
            nc.scalar.copy(E28s, Ep28)
            Ep = fin_ps.tile([NT, 1], f32, tag="f", name="Ep")
            nc.tensor.matmul(Ep, lhsT=sum4x, rhs=E28s, start=True, stop=False)
            nc.tensor.matmul(Ep, lhsT=c47, rhs=icpt, start=False, stop=True)
            outs = psb.tile([NT, 1], f32, tag="outs", name="outs")
            nc.scalar.copy(outs, Ep)
            nc.sync.dma_start(out=t["out"][:, :], in_=outs)


def _in_maps(inputs):
    import ml_dtypes
    f = np.float32
    bf = ml_dtypes.bfloat16
    c = np.ascontiguousarray
    h1, h2 = inputs["h1"], inputs["h2"]
    adj1, adj2 = inputs["adj1"], inputs["adj2"]
    A_int, dmv, valid = inputs["A_int"], inputs["dmv"], inputs["valid"]
    WA1 = np.asarray(inputs["WA1"], f).reshape(NT, 2, D, H)
    WB1 = np.asarray(inputs["WB1"], f).reshape(NT, 2, D, H)
    gWb = np.asarray(inputs["gWb"], f)          # [L, D]
    gW_ = np.asarray(inputs["gW"], f)           # [L, D, D]
    gA_ = np.asarray(inputs["gA"], f)

    cpack = np.zeros((128, C_TOT), f)
    cpack[:, C_BA1:C_BA1 + NT] = np.asarray(inputs["bA1"], f).T
    cpack[:, C_BB1:C_BB1 + NT] = np.asarray(inputs["bB1"], f).T
    cpack[:, C_NEGC:C_NEGC + NT] = -np.asarray(inputs["C"], f).reshape(1, NT)
    cpack[:, C_BA2:C_BA2 + NT] = np.asarray(inputs["bA2"], f).reshape(1, NT)
    cpack[:, C_BB2H:C_BB2H + NT] = (np.asarray(inputs["bB2"], f)
                                    .reshape(1, NT) * 0.5)
    cpack[:, C_WI1:C_WI1 + H] = np.asarray(inputs["Wi1"], f)
    cpack[:, C_BI1] = np.asarray(inputs["bi1"], f).reshape(H)
    cpack[:, C_WI2] = np.asarray(inputs["Wi2"], f).reshape(H)
    cpack[0, C_BI2] = -np.asarray(inputs["bi2"], f).reshape(1)[0]
    cpack[:, C_EPS] = 1e-10
    cpack[:, C_GWB:C_GWB + L] = gWb.T
    cpack[:, C_UB:C_UB + L] = np.einsum('lij,li->lj', gA_, gWb).T

    gatpk = np.zeros((128, G_TOT), bf)
    gatpk[:, G_GW:G_GW + L * D] = gW_.transpose(1, 0, 2).reshape(D, L * D)
    gAW = np.einsum('lij,ljk->lik', gW_, gA_)
    gatpk[:, G_GAW:G_GAW + L * D] = gAW.transpose(1, 0, 2).reshape(D, L * D)
    gatpk[:, G_GGW:G_GGW + 2 * L] = (np.asarray(inputs["gGateW"], f)
                                     .reshape(L, 2, D).transpose(2, 0, 1)
                                     .reshape(D, 2 * L))
    gatpk[:, G_EYE:G_EYE + 128] = np.eye(128, dtype=f)
    gatpk[:, G_EYEK:G_EYEK + 128] = np.eye(128, dtype=f) * BIG

    w1pk = np.zeros((128, W_TOT), bf)
    w1pk[:, W_A:W_A + NT * 2 * H] = WA1.transpose(2, 0, 1, 3).reshape(
        D, NT * 2 * H)
    w1pk[:, W_B:W_B + NT * 2 * H] = WB1.transpose(2, 0, 1, 3).reshape(
        D, NT * 2 * H)
    w1pk[:, W_W2A:W_W2A + NT] = np.asarray(inputs["WA2"], f).T
    w1pk[:, W_W2B:W_W2B + NT] = np.asarray(inputs["WB2"], f).T

    p1 = np.zeros((1, 10), f)
    p1[0, 0:L] = np.asarray(inputs["gGateb"], f).reshape(L) * 0.5

    sum4x = np.repeat(np.eye(NT, dtype=f), 4, axis=0) * 4.0

    shared = {
        "cpack": cpack, "gatpk": gatpk, "w1pk": w1pk, "p1": p1,
        "sum4x": sum4x,
    }
    maps = []
    for b in range(B):
        m = dict(shared)
        pk56 = np.zeros((F_IN, N1 + N2 + D), bf)
        pk56[:, 0:N1] = h1[b].T
        pk56[:, N1:N1 + N2] = h2[b].T
        pk56[:, N1 + N2:] = np.asarray(inputs["W_embed"], f)
        m["pk56"] = pk56
        m["dmvT"] = c(dmv[b].transpose(1, 0, 2).reshape(4, 128, N1 * 3)
                      .transpose(1, 0, 2).reshape(128, 4 * N1 * 3), dtype=f)
        m["mb2"] = c(adj2[b].T.reshape(4, 128, N2).transpose(1, 0, 2)
                     .reshape(128, 4 * N2) - 1.0).astype(bf)
        m["mb1"] = c(adj1[b].T - 1.0).astype(bf)
        m["aint"] = c(A_int[b].transpose(2, 0, 1).reshape(4, 128, NT, N1)
                      .transpose(1, 2, 0, 3).reshape(128, NT * 4 * N1)
                      ).astype(bf)
        m["valid"] = c(valid[b].reshape(N1, 1), dtype=f)
        maps.append(m)
    return maps


def _make_runner(nc, n_cores):
    """Persistent jitted SPMD runner (mirrors bass2jax.run_bass_via_pjrt but
    caches the compiled executable so repeat calls don't re-lower)."""
    import jax
    import concourse.mybir as mybir_
    from concourse import bass2jax
    from jax.experimental.shard_map import shard_map
    from jax.sharding import Mesh, PartitionSpec

    bass2jax.install_neuronx_cc_hook()
    partition_name = (nc.partition_id_tensor.name
                      if nc.partition_id_tensor else None)
    in_names, out_names, out_avals, zero_outs = [], [], [], []
    for alloc in nc.m.functions[0].allocations:
        if not isinstance(alloc, mybir_.MemoryLocationSet):
            continue
        name = alloc.memorylocations[0].name
        if alloc.kind == "ExternalInput":
            if name != partition_name:
                in_names.append(name)
        elif alloc.kind == "ExternalOutput":
            shape = tuple(alloc.tensor_shape)
            dtype = mybir_.dt.np(alloc.dtype)
            out_names.append(name)
            out_avals.append(jax.core.ShapedArray(shape, dtype))
            zero_outs.append(np.zeros(shape, dtype))
    n_params = len(in_names)
    n_outs = len(out_avals)
    all_in = list(in_names) + list(out_names)
    if partition_name is not None:
        all_in.append(partition_name)
    donate = tuple(range(n_params, n_params + n_outs))

    def _body(*args):
        operands = list(args)
        if partition_name is not None:
            operands.append(bass2jax.partition_id_tensor())
        outs = bass2jax._bass_exec_p.bind(
            *operands,
            out_avals=tuple(out_avals),
            in_names=tuple(all_in),
            out_names=tuple(out_names),
            lowering_input_output_aliases=(),
            sim_require_finite=True,
            sim_require_nnan=True,
            nc=nc,
        )
        return tuple(outs)

    devices = jax.devices()[:n_cores]
    mesh = Mesh(np.asarray(devices), ("core",))
    sharded = jax.jit(
        shard_map(_body, mesh=mesh,
                  in_specs=(PartitionSpec("core"),) * (n_params + n_outs),
                  out_specs=(PartitionSpec("core"),) * n_outs,
                  check_rep=False),
        donate_argnums=donate, keep_unused=True)

    def run(in_maps, timing_reps=0):
        concat_in = [
            np.concatenate([np.asarray(m[name]) for m in in_maps], axis=0)
            for name in in_names
        ]
        concat_zeros = [
            np.zeros((n_cores * z.shape[0], *z.shape[1:]), z.dtype)
            for z in zero_outs
        ]
        out_arrs = sharded(*concat_in, *concat_zeros)
        out_arrs = [np.asarray(a) for a in out_arrs]
        if timing_reps:
            import time
            from jax.sharding import NamedSharding
            shard = NamedSharding(mesh, PartitionSpec("core"))
            dev_in = [jax.device_put(x, shard) for x in concat_in]
            jax.block_until_ready(dev_in)

            def one():
                zs = [np.zeros((n_cores * z.shape[0], *z.shape[1:]), z.dtype)
                      for z in zero_outs]
                return sharded(*dev_in, *zs)

            jax.block_until_ready(one())
            times = []
            for _ in range(timing_reps):
                t0 = time.perf_counter()
                r = one()
                jax.block_until_ready(r)
                times.append(time.perf_counter() - t0)
            times.sort()
            LAST_RESULT["wall_per_call_s"] = times[0]
            LAST_RESULT["wall_median_s"] = times[len(times) // 2]
            LAST_RESULT["wall_all"] = times
        return [
            {name: out_arrs[i].reshape(n_cores, *out_avals[i].shape)[c]
             for i, name in enumerate(out_names)}
            for c in range(n_cores)
        ]

    return run


def kernel(**inputs):
    inputs = {k: np.asarray(v) for k, v in inputs.items()}
    if "nc" not in _cache:
        _cache["nc"] = _build()
        _cache["run"] = _make_runner(_cache["nc"], B)
    in_maps = _in_maps(inputs)
    results = _cache["run"](in_maps, timing_reps=TIMING_REPS)
    out = np.stack([results[b]["out"][:, 0] for b in range(B)], axis=0)
    return out.astype(np.float32)
